# revision 13
# baseline (speedup 1.0000x reference)
"""Trainium2 Bass kernel for nn_DynamycMoE (dense-masked top-2 MoE).

Strategy (MODE="ep"): expert-parallel in two SPMD launches.
  Phase 1 (data-parallel, fp32): each of the 8 cores computes top-2
  softmax gates for its 1024-token shard. Gating stays fully fp32: the
  top-2 *selection* is discontinuous, and bf16 logits flip selections on
  near-tied tokens, producing O(1) output errors. x-tile loads are split
  across the SP and Activation HWDGE queues so the transfers pipeline.
  Host dispatch: tokens are gathered per expert id (gate > 0), padded to
  a static capacity NCAP.
  Phase 2 (expert-parallel, bf16): core e runs expert e's MLP on its
  gathered tokens, activations token-on-free-axis:
      hT = relu(W1t.T @ xT + b1)        PSUM f32 -> SBUF bf16
      oT = W2t.T @ hT                   PSUM f32
      og = (oT + b2) * gate_bcast       -> SBUF bf16
      y  = og.T @ WmT                   PSUM f32 -> SBUF bf16 -> HBM
  bf16 matmuls run at 1 cycle/row (fp32 is 4) and halve DMA bytes; the
  smooth expert pipeline tolerates bf16 (rel err ~4.5e-3 vs the 2e-2
  gate; top-2 selection exactness is what matters and that is fp32).
  Host combines the two gated expert outputs per token in ascending
  expert order and applies the eps substitution.

Fallback (MODE="dp" or capacity overflow): fully-fused dense-masked MoE,
data-parallel over tokens (slow but always correct).
"""

import ml_dtypes
import numpy as np

import concourse.bacc as bacc
import concourse.bass as bass
import concourse.mybir as mybir
import concourse.tile as tile
from concourse import bass_utils

F32 = mybir.dt.float32
BF16 = mybir.dt.bfloat16
NP_BF16 = ml_dtypes.bfloat16
AF = mybir.ActivationFunctionType
ALU = mybir.AluOpType

B, D, H, E, C, T = 8192, 768, 256, 8, 64, 512
NCORES = 8
BL = B // NCORES  # tokens per core
TT = 256          # gate phase: token tile (free-dim) size
NT = BL // TT     # gate phase: token tiles per core
DC = D // 128     # K-chunks over D
HC = H // 128     # K-chunks over H
NPAIR = E // 2
EPS = float(np.finfo(np.float64).eps)

MODE = "ep"        # "ep": expert-parallel 2-phase; "dp": data-parallel dense
NCAP = 2304        # EP: padded per-expert token capacity (mean load 2048)
TTE = 384          # EP: token tile size (3 q-chunks of 128)
NTE = NCAP // TTE
QT = TTE // 128


def _build_nc(reps=1):
    """Dense-masked data-parallel fallback (fp32, slow, always correct)."""
    nc = bacc.Bacc(
        "TRN2", target_bir_lowering=False, debug=False, enable_asserts=False
    )

    xT_h = nc.dram_tensor("xT", [128, NT * DC * TT], F32, kind="ExternalInput")
    wg_h = nc.dram_tensor("wg", [128, DC * E], F32, kind="ExternalInput")
    w1_h = nc.dram_tensor("w1", [128, E * DC * H], F32, kind="ExternalInput")
    b1_h = nc.dram_tensor("b1", [128, E * HC], F32, kind="ExternalInput")
    w2_h = nc.dram_tensor("w2", [128, E * HC * C], F32, kind="ExternalInput")
    b2_h = nc.dram_tensor("b2", [64, E], F32, kind="ExternalInput")
    wm_h = nc.dram_tensor("wm", [128, NPAIR * T], F32, kind="ExternalInput")
    id_h = nc.dram_tensor("ident", [128, 128], F32, kind="ExternalInput")
    y_h = nc.dram_tensor("y", [BL, T], F32, kind="ExternalOutput")

    w1_v = w1_h[:].rearrange("p (e c h) -> p e c h", e=E, c=DC)
    xT_v = xT_h[:].rearrange("p (i c t) -> p i c t", i=NT, c=DC)

    with tile.TileContext(nc) as tc:
        with (
            tc.tile_pool(name="weights", bufs=1) as wpool,
            tc.tile_pool(name="gates", bufs=1) as gpool,
            tc.tile_pool(name="gtmp", bufs=2) as gtmp,
            tc.tile_pool(name="hsb", bufs=3) as hpool,
            tc.tile_pool(name="og", bufs=3) as ogpool,
            tc.tile_pool(name="gb", bufs=4) as gbpool,
            tc.tile_pool(name="yout", bufs=4) as ypool,
        ):
            wg = wpool.tile([128, DC, E], F32, tag="wg")
            nc.sync.dma_start(wg[:], wg_h[:].rearrange("p (c e) -> p c e", c=DC))
            xts = []
            for ti in range(NT):
                xt = wpool.tile([128, DC, TT], F32, tag=f"x{ti}")
                nc.sync.dma_start(xt[:], xT_v[:, ti, :, :])
                xts.append(xt)
            w1s = []
            for e in range(E):
                w1e = wpool.tile([128, DC, H], F32, tag=f"w1_{e}")
                nc.sync.dma_start(w1e[:], w1_v[:, e, :, :])
                w1s.append(w1e)
            b1 = wpool.tile([128, E, HC], F32, tag="b1")
            nc.sync.dma_start(b1[:], b1_h[:].rearrange("p (e c) -> p e c", e=E))
            w2 = wpool.tile([128, E, HC, C], F32, tag="w2")
            nc.sync.dma_start(
                w2[:], w2_h[:].rearrange("p (e c k) -> p e c k", e=E, c=HC)
            )
            b2 = wpool.tile([64, E], F32, tag="b2")
            nc.sync.dma_start(b2[:], b2_h[:])
            wm = wpool.tile([128, NPAIR, T], F32, tag="wm")
            nc.sync.dma_start(wm[:], wm_h[:].rearrange("p (g t) -> p g t", g=NPAIR))
            ident = wpool.tile([128, 128], F32, tag="ident")
            nc.sync.dma_start(ident[:], id_h[:])

            for _ in range(reps):
                gflats = []
                with tc.tile_pool(
                    name="ps_gate", bufs=2, space=bass.MemorySpace.PSUM
                ) as ps_g:
                    for ti in range(NT):
                        gatesT = gpool.tile([8, TT], F32, tag=f"gatesT{ti}")
                        for qq in range(TT // 128):
                            tok = qq * 128
                            lg = ps_g.tile([128, E], F32, tag="lg")
                            for kc in range(DC):
                                nc.tensor.matmul(
                                    lg[:],
                                    xts[ti][:, kc, tok : tok + 128],
                                    wg[:, kc, :],
                                    start=(kc == 0),
                                    stop=(kc == DC - 1),
                                )
                            mx1 = gtmp.tile([128, 1], F32, tag="mx1")
                            nc.vector.reduce_max(
                                mx1[:], lg[:], axis=mybir.AxisListType.X
                            )
                            is1 = gtmp.tile([128, E], F32, tag="is1")
                            nc.vector.tensor_scalar(
                                is1[:], lg[:], mx1[:], None, op0=ALU.is_equal
                            )
                            masked = gtmp.tile([128, E], F32, tag="masked")
                            nc.vector.scalar_tensor_tensor(
                                masked[:],
                                is1[:],
                                -1e30,
                                lg[:],
                                op0=ALU.mult,
                                op1=ALU.add,
                            )
                            mx2 = gtmp.tile([128, 1], F32, tag="mx2")
                            nc.vector.reduce_max(
                                mx2[:], masked[:], axis=mybir.AxisListType.X
                            )
                            is2 = gtmp.tile([128, E], F32, tag="is2")
                            nc.vector.tensor_scalar(
                                is2[:], masked[:], mx2[:], None, op0=ALU.is_equal
                            )
                            d = gtmp.tile([128, 1], F32, tag="d")
                            nc.vector.tensor_sub(d[:], mx2[:], mx1[:])
                            ed = gtmp.tile([128, 1], F32, tag="ed")
                            nc.scalar.activation(ed[:], d[:], AF.Exp)
                            den = gtmp.tile([128, 1], F32, tag="den")
                            nc.vector.tensor_scalar_add(den[:], ed[:], 1.0)
                            g1 = gtmp.tile([128, 1], F32, tag="g1")
                            nc.vector.reciprocal(g1[:], den[:])
                            g2 = gtmp.tile([128, 1], F32, tag="g2")
                            nc.vector.tensor_mul(g2[:], ed[:], g1[:])
                            t2 = gtmp.tile([128, E], F32, tag="t2")
                            nc.vector.tensor_scalar_mul(t2[:], is2[:], g2[:])
                            gq = gtmp.tile([128, E], F32, tag="gq")
                            nc.vector.scalar_tensor_tensor(
                                gq[:], is1[:], g1[:], t2[:], op0=ALU.mult, op1=ALU.add
                            )
                            tr = ps_g.tile([8, 128], F32, tag="tr")
                            nc.tensor.transpose(tr[:], gq[:], ident[:])
                            nc.vector.tensor_copy(gatesT[:, tok : tok + 128], tr[:])
                        gflat = gpool.tile([1, E, TT], F32, tag=f"gflat{ti}")
                        nc.sync.dma_start(gflat[0:1, :, :], gatesT[:, :])
                        gflats.append(gflat)

                with (
                    tc.tile_pool(
                        name="ps_h", bufs=2, space=bass.MemorySpace.PSUM
                    ) as ps_h,
                    tc.tile_pool(
                        name="ps_o", bufs=2, space=bass.MemorySpace.PSUM
                    ) as ps_o,
                    tc.tile_pool(
                        name="ps_y", bufs=2, space=bass.MemorySpace.PSUM
                    ) as ps_y,
                ):
                    for ti in range(NT):
                        y_ps = ps_y.tile([128, TT // 128, T], F32, tag="y")
                        for pair in range(NPAIR):
                            og = ogpool.tile([128, TT], F32, tag="og")
                            for j in range(2):
                                e = 2 * pair + j
                                hT = ps_h.tile([128, HC, TT], F32, tag="h")
                                for half in range(HC):
                                    for kc in range(DC):
                                        nc.tensor.matmul(
                                            hT[:, half, :],
                                            w1s[e][:, kc, half * 128 : half * 128 + 128],
                                            xts[ti][:, kc, :],
                                            start=(kc == 0),
                                            stop=(kc == DC - 1),
                                        )
                                hs = hpool.tile([128, HC, TT], F32, tag="hs")
                                for half in range(HC):
                                    nc.scalar.activation(
                                        hs[:, half, :],
                                        hT[:, half, :],
                                        AF.Relu,
                                        bias=b1[:, e, half : half + 1],
                                    )
                                oT = ps_o.tile([64, TT], F32, tag="o")
                                for kc in range(HC):
                                    nc.tensor.matmul(
                                        oT[:],
                                        w2[:, e, kc, :],
                                        hs[:, kc, :],
                                        start=(kc == 0),
                                        stop=(kc == HC - 1),
                                    )
                                gb = gbpool.tile([64, TT], F32, tag="gb")
                                nc.gpsimd.partition_broadcast(
                                    gb[:], gflats[ti][0:1, e, :]
                                )
                                nc.vector.scalar_tensor_tensor(
                                    og[j * 64 : j * 64 + 64, :],
                                    oT[:],
                                    b2[:, e : e + 1],
                                    gb[:],
                                    op0=ALU.add,
                                    op1=ALU.mult,
                                )
                            for q in range(TT // 128):
                                nc.tensor.matmul(
                                    y_ps[:, q, :],
                                    og[:, q * 128 : q * 128 + 128],
                                    wm[:, pair, :],
                                    start=(pair == 0),
                                    stop=(pair == NPAIR - 1),
                                )
                        for q in range(TT // 128):
                            tok = ti * TT + q * 128
                            mask = ypool.tile([128, T], F32, tag="mask")
                            nc.vector.tensor_scalar(
                                mask[:], y_ps[:, q, :], 0.0, None, op0=ALU.is_equal
                            )
                            ysb = ypool.tile([128, T], F32, tag="ysb")
                            nc.vector.scalar_tensor_tensor(
                                ysb[:],
                                mask[:],
                                EPS,
                                y_ps[:, q, :],
                                op0=ALU.mult,
                                op1=ALU.add,
                            )
                            nc.sync.dma_start(y_h[tok : tok + 128, :], ysb[:])

    nc.compile()
    return nc


def _build_gate_nc(reps=1):
    """EP phase 1: per-core fp32 gating over its 1024-token shard.

    x-tile loads alternate between the SP and Activation HWDGE queues so
    the four 768KB transfers pipeline instead of serializing behind one
    queue's issue overhead. Gating math is batched across all 8 token
    chunks: one [128, 8, 8] (partition, chunk, expert) PSUM tile.
    """
    QC = BL // 128
    nc = bacc.Bacc(
        "TRN2", target_bir_lowering=False, debug=False, enable_asserts=False
    )
    xT_h = nc.dram_tensor("xT", [128, NT * DC * TT], F32, kind="ExternalInput")
    wg_h = nc.dram_tensor("wg", [128, DC * E], F32, kind="ExternalInput")
    g_h = nc.dram_tensor("gates", [128, QC * E], F32, kind="ExternalOutput")
    xT_v = xT_h[:].rearrange("p (i c t) -> p i c t", i=NT, c=DC)

    with tile.TileContext(nc) as tc:
        with (
            tc.tile_pool(name="weights", bufs=1) as wpool,
            tc.tile_pool(name="gtmp", bufs=2) as gtmp,
            tc.tile_pool(name="ps_g", bufs=2, space=bass.MemorySpace.PSUM) as ps_g,
        ):
            wg = wpool.tile([128, DC, E], F32, tag="wg")
            nc.scalar.dma_start(wg[:], wg_h[:].rearrange("p (c e) -> p c e", c=DC))
            # x in two half-shard transfers on separate HWDGE queues: one
            # transfer per queue avoids the per-DMA DGE bubble entirely.
            xall = wpool.tile([128, NT, DC, TT], F32, tag="xall")
            nc.sync.dma_start(xall[:, 0 : NT // 2], xT_v[:, 0 : NT // 2, :, :])
            nc.scalar.dma_start(xall[:, NT // 2 :], xT_v[:, NT // 2 :, :, :])
            xts = [xall[:, ti] for ti in range(NT)]

            for _ in range(reps):
                lg = ps_g.tile([128, QC, E], F32, tag="lg")
                for q in range(QC):
                    ti, sub = q // (TT // 128), q % (TT // 128)
                    for kc in range(DC):
                        nc.tensor.matmul(
                            lg[:, q, :],
                            xts[ti][:, kc, sub * 128 : sub * 128 + 128],
                            wg[:, kc, :],
                            start=(kc == 0),
                            stop=(kc == DC - 1),
                        )

                def b3(ap2d):  # [128, QC] -> [128, QC, E] free-broadcast
                    return ap2d.unsqueeze(2).to_broadcast([128, QC, E])

                mx1 = gtmp.tile([128, QC], F32, tag="mx1")
                nc.vector.reduce_max(mx1[:], lg[:], axis=mybir.AxisListType.X)
                is1 = gtmp.tile([128, QC, E], F32, tag="is1")
                nc.vector.tensor_tensor(is1[:], lg[:], b3(mx1[:]), op=ALU.is_equal)
                masked = gtmp.tile([128, QC, E], F32, tag="masked")
                nc.vector.scalar_tensor_tensor(
                    masked[:], is1[:], -1e30, lg[:], op0=ALU.mult, op1=ALU.add
                )
                mx2 = gtmp.tile([128, QC], F32, tag="mx2")
                nc.vector.reduce_max(mx2[:], masked[:], axis=mybir.AxisListType.X)
                is2 = gtmp.tile([128, QC, E], F32, tag="is2")
                nc.vector.tensor_tensor(is2[:], masked[:], b3(mx2[:]), op=ALU.is_equal)
                d = gtmp.tile([128, QC], F32, tag="d")
                nc.vector.tensor_sub(d[:], mx2[:], mx1[:])
                ed = gtmp.tile([128, QC], F32, tag="ed")
                nc.scalar.activation(ed[:], d[:], AF.Exp)
                den = gtmp.tile([128, QC], F32, tag="den")
                nc.vector.tensor_scalar_add(den[:], ed[:], 1.0)
                g1 = gtmp.tile([128, QC], F32, tag="g1")
                nc.vector.reciprocal(g1[:], den[:])
                g2 = gtmp.tile([128, QC], F32, tag="g2")
                nc.vector.tensor_mul(g2[:], ed[:], g1[:])
                t2 = gtmp.tile([128, QC, E], F32, tag="t2")
                nc.vector.tensor_tensor(t2[:], is2[:], b3(g2[:]), op=ALU.mult)
                u1 = gtmp.tile([128, QC, E], F32, tag="u1")
                nc.vector.tensor_tensor(u1[:], is1[:], b3(g1[:]), op=ALU.mult)
                gq = gtmp.tile([128, QC, E], F32, tag="gq")
                nc.vector.tensor_add(gq[:], u1[:], t2[:])
                # contiguous [128, QC*E] write (host un-permutes for free);
                # a (q p) e scatter would cost 1024 tiny descriptors.
                nc.sync.dma_start(
                    g_h[:].rearrange("p (q e) -> p q e", q=QC), gq[:]
                )
    nc.compile()
    return nc


def _build_ep_nc(reps=1):
    """EP phase 2 (bf16): one expert per core over NCAP gathered tokens.

    All matmul operands are bf16 (1 cycle/row vs fp32's 4); PSUM stays
    f32. Inputs stream on the SP queue in first-use order; per-tile
    outputs are batched into one SWDGE DMA on the Pool queue. PSUM->bf16
    converts are split across the Activation and Vector engines so no
    single engine exceeds the PE's per-tile budget.
    """
    nc = bacc.Bacc(
        "TRN2", target_bir_lowering=False, debug=False, enable_asserts=False
    )
    xg_h = nc.dram_tensor("xg", [128, NTE * DC * TTE], BF16, kind="ExternalInput")
    w1_h = nc.dram_tensor("w1", [128, HC * DC * 128], BF16, kind="ExternalInput")
    b1_h = nc.dram_tensor("b1", [128, HC], F32, kind="ExternalInput")
    w2_h = nc.dram_tensor("w2", [128, HC * C], BF16, kind="ExternalInput")
    b2_h = nc.dram_tensor("b2", [64, 1], F32, kind="ExternalInput")
    wm_h = nc.dram_tensor("wm", [64, T], BF16, kind="ExternalInput")
    gr_h = nc.dram_tensor("grow", [1, NCAP], F32, kind="ExternalInput")
    yp_h = nc.dram_tensor("yp", [NCAP, T], BF16, kind="ExternalOutput")
    xg_v = xg_h[:].rearrange("p (i c t) -> p i c t", i=NTE, c=DC)
    w1_v = w1_h[:].rearrange("p (f c h) -> p f c h", f=HC, c=DC)

    with tile.TileContext(nc) as tc:
        with (
            tc.tile_pool(name="weights", bufs=1) as wpool,
            tc.tile_pool(name="hsb", bufs=3) as hpool,
            tc.tile_pool(name="og", bufs=3) as ogpool,
            tc.tile_pool(name="gb", bufs=3) as gbpool,
            tc.tile_pool(name="yout", bufs=3) as ypool,
            tc.tile_pool(name="ps_h", bufs=2, space=bass.MemorySpace.PSUM) as ps_h,
            tc.tile_pool(name="ps_o", bufs=2, space=bass.MemorySpace.PSUM) as ps_o,
            tc.tile_pool(name="ps_y", bufs=2, space=bass.MemorySpace.PSUM) as ps_y,
        ):
            # SP queue carries the critical-path stream in first-use order:
            # W1 half-0 slab, xg tile 0, W1 half-1, then grouped xg tiles
            # (grouping amortizes the ~650ns per-DMA DGE bubble). The small
            # params ride the Activation queue and all land before first use.
            w1all = wpool.tile([128, HC, DC, 128], BF16, tag="w1")
            xgall = wpool.tile([128, NTE, DC, TTE], BF16, tag="xg")
            nc.sync.dma_start(w1all[:, 0:1], w1_v[:, 0:1, :, :])
            nc.sync.dma_start(xgall[:, 0:1], xg_v[:, 0:1, :, :])
            nc.sync.dma_start(w1all[:, 1:2], w1_v[:, 1:2, :, :])
            nc.sync.dma_start(xgall[:, 1:3], xg_v[:, 1:3, :, :])
            nc.sync.dma_start(xgall[:, 3:6], xg_v[:, 3:6, :, :])
            w1h = [w1all[:, half] for half in range(HC)]
            xgs = [xgall[:, ti] for ti in range(NTE)]

            b1 = wpool.tile([128, HC], F32, tag="b1")
            nc.scalar.dma_start(b1[:], b1_h[:])
            grow = wpool.tile([1, NCAP], F32, tag="grow")
            nc.scalar.dma_start(grow[:], gr_h[:])
            w2 = wpool.tile([128, HC, C], BF16, tag="w2")
            nc.scalar.dma_start(w2[:], w2_h[:].rearrange("p (c k) -> p c k", c=HC))
            b2 = wpool.tile([64, 1], F32, tag="b2")
            nc.scalar.dma_start(b2[:], b2_h[:])
            wm = wpool.tile([64, T], BF16, tag="wm")
            nc.scalar.dma_start(wm[:], wm_h[:])

            # PE p-state prewarm: dependent dummy matmuls on a zeroed tile
            # keep the PE busy through the input-DMA lead-in so the 3us
            # frequency ramp (0.65 -> 1.2 -> 2.4 GHz) completes before the
            # first real matmul. Results land in a scratch PSUM tile that
            # is never read.
            warm = wpool.tile([128, 128], BF16, tag="warm")
            nc.gpsimd.memset(warm[:], 0)
            wps = ps_h.tile([128, TTE], F32, tag="h0")
            for _ in range(18):
                nc.tensor.matmul(
                    wps[:, 0:128], warm[:], warm[:], start=True, stop=True
                )

            # whole-capacity gate broadcast, once: keeps the Pool engine out
            # of the per-tile dependency chain (its queue also carries the
            # batched output DMAs).
            gball = gbpool.tile([64, NCAP], F32, tag="gball")
            nc.gpsimd.partition_broadcast(gball[:], grow[0:1, :])

            for rr in range(reps):
                for ti in range(NTE):
                    hs = hpool.tile([128, HC, TTE], BF16, tag="hs")
                    hT0 = ps_h.tile([128, TTE], F32, tag="h0")
                    hT1 = ps_h.tile([128, TTE], F32, tag="h1")
                    hTs = [hT0, hT1]
                    # interleave the two half-chains so PE always has an
                    # independent matmul to issue between dependent
                    # accumulation steps.
                    for kc in range(DC):
                        for half in range(HC):
                            nc.tensor.matmul(
                                hTs[half][:],
                                w1h[half][:, kc, :],
                                xgs[ti][:, kc, :],
                                start=(kc == 0),
                                stop=(kc == DC - 1),
                            )
                    for half in range(HC):
                        nc.scalar.activation(
                            hs[:, half, :],
                            hTs[half][:],
                            AF.Relu,
                            bias=b1[:, half : half + 1],
                        )
                    oT = ps_o.tile([64, TTE], F32, tag="o")
                    for kc in range(HC):
                        nc.tensor.matmul(
                            oT[:],
                            w2[:, kc, :],
                            hs[:, kc, :],
                            start=(kc == 0),
                            stop=(kc == HC - 1),
                        )
                    og = ogpool.tile([64, TTE], BF16, tag="og")
                    nc.vector.scalar_tensor_tensor(
                        og[:],
                        oT[:],
                        b2[:, 0:1],
                        gball[:, ti * TTE : ti * TTE + TTE],
                        op0=ALU.add,
                        op1=ALU.mult,
                    )
                    ysb = ypool.tile([128, QT, T], BF16, tag="ysb")
                    last = ti == NTE - 1
                    for q in range(QT):
                        y_ps = ps_y.tile([128, T], F32, tag="y")
                        nc.tensor.matmul(
                            y_ps[:],
                            og[:, q * 128 : q * 128 + 128],
                            wm[:],
                            start=True,
                            stop=True,
                        )
                        # spread the PSUM->bf16 converts: Act gets 2 on even
                        # tiles, DVE 2 on odd tiles (plus its og op).
                        if (q + ti) % 2 == 0:
                            nc.scalar.copy(ysb[:, q, :], y_ps[:])
                        else:
                            nc.vector.tensor_copy(ysb[:, q, :], y_ps[:])
                        if last:
                            # final tile: per-chunk writes on three queues so
                            # the tail is one small transfer, not a batched
                            # SWDGE generation + 384-descriptor DMA.
                            eng = (nc.gpsimd, nc.sync, nc.scalar)[q]
                            tok = ti * TTE + q * 128
                            eng.dma_start(
                                yp_h[tok : tok + 128, :], ysb[:, q, :]
                            )
                    if not last:
                        nc.gpsimd.dma_start(
                            yp_h[ti * TTE : ti * TTE + TTE, :].rearrange(
                                "(q p) t -> p q t", p=128
                            ),
                            ysb[:],
                        )
    nc.compile()
    return nc


_NC_CACHE = {}


def _get_nc(which="dp"):
    if which not in _NC_CACHE:
        _NC_CACHE[which] = {
            "dp": _build_nc,
            "gate": _build_gate_nc,
            "ep": _build_ep_nc,
        }[which]()
    return _NC_CACHE[which]


def _host_prep(x, w_gate, W1, b1, W2, b2, Wm):
    """Dense fallback: rearrange weights into SBUF images; shard x."""
    f = np.float32
    xs = []
    for c in range(NCORES):
        s = x[c * BL : (c + 1) * BL]  # [BL, D]
        img = np.ascontiguousarray(
            s.reshape(NT, TT, DC, 128).transpose(3, 0, 2, 1).reshape(128, -1)
        )
        xs.append(img)
    W1t = W1.transpose(0, 2, 1)  # [E, D, H]
    w1_img = np.ascontiguousarray(
        W1t.reshape(E, DC, 128, H).transpose(2, 0, 1, 3).reshape(128, -1)
    )
    W2t = W2.transpose(0, 2, 1)  # [E, H, C]
    w2_img = np.ascontiguousarray(
        W2t.reshape(E, HC, 128, C).transpose(2, 0, 1, 3).reshape(128, -1)
    )
    WmT = Wm.transpose(0, 2, 1)  # [E, C, T]
    wm_img = np.ascontiguousarray(
        WmT.reshape(NPAIR, 128, T).transpose(1, 0, 2).reshape(128, -1)
    )
    wg_img = np.ascontiguousarray(
        w_gate.reshape(DC, 128, E).transpose(1, 0, 2).reshape(128, -1)
    )
    b1_img = np.ascontiguousarray(
        b1.reshape(E, HC, 128).transpose(2, 0, 1).reshape(128, -1)
    )
    b2_img = np.ascontiguousarray(b2.T)  # [C, E]
    ident = np.eye(128, dtype=f)
    shared = {
        "wg": wg_img.astype(f, copy=False),
        "w1": w1_img.astype(f, copy=False),
        "b1": b1_img.astype(f, copy=False),
        "w2": w2_img.astype(f, copy=False),
        "b2": b2_img.astype(f, copy=False),
        "wm": wm_img.astype(f, copy=False),
        "ident": ident,
    }
    return [dict(shared, xT=xs[c].astype(f, copy=False)) for c in range(NCORES)]


def _x_images(x):
    """Per-core feature-major fp32 SBUF images of the token shards."""
    xs = []
    for c in range(NCORES):
        s = x[c * BL : (c + 1) * BL]
        xs.append(
            np.ascontiguousarray(
                s.reshape(NT, TT, DC, 128).transpose(3, 0, 2, 1).reshape(128, -1)
            )
        )
    return xs


def _wg_image(w_gate):
    return np.ascontiguousarray(
        w_gate.reshape(DC, 128, E).transpose(1, 0, 2).reshape(128, -1)
    )


def _make_ep_map(xg, grow, W1e, b1e, W2e, b2e, Wme):
    """Build the bf16 phase-2 in_map for one expert.

    xg: [NCAP, D] f32 (gathered+padded tokens), grow: [1, NCAP] f32.
    """
    bf = NP_BF16
    xg_img = np.ascontiguousarray(
        xg.reshape(NTE, TTE, DC, 128).transpose(3, 0, 2, 1).reshape(128, -1)
    ).astype(bf)
    # half-major [p, half, kc, h'] to match the split W1 slab tiles
    w1_img = np.ascontiguousarray(
        W1e.T.reshape(DC, 128, HC, 128).transpose(1, 2, 0, 3).reshape(128, -1)
    ).astype(bf)
    w2_img = np.ascontiguousarray(
        W2e.T.reshape(HC, 128, C).transpose(1, 0, 2).reshape(128, -1)
    ).astype(bf)
    return {
        "xg": xg_img,
        "w1": w1_img,
        "b1": np.ascontiguousarray(b1e.reshape(HC, 128).T),
        "w2": w2_img,
        "b2": np.ascontiguousarray(b2e[:, None]),
        "wm": np.ascontiguousarray(Wme.T).astype(bf),  # [C, T]
        "grow": grow,
    }


def _kernel_ep(x, w_gate, W1, b1, W2, b2, Wm):
    # phase 1: on-device fp32 gating
    xs = _x_images(x)
    wg_img = _wg_image(w_gate)
    g_maps = [{"xT": xs[c], "wg": wg_img} for c in range(NCORES)]
    nc_g = _get_nc("gate")
    res_g = bass_utils.run_bass_kernel_spmd(nc_g, g_maps, list(range(NCORES)))
    QC = BL // 128
    gates = np.concatenate(
        [
            res_g.results[c]["gates"]
            .reshape(128, QC, E)
            .transpose(1, 0, 2)
            .reshape(BL, E)
            for c in range(NCORES)
        ],
        axis=0,
    )  # [B, E]; device writes [p, q, e], token = q*128 + p

    # host dispatch: gather tokens per expert (top-2 membership = gate > 0)
    idxs, ep_maps = [], []
    for e in range(E):
        idx = np.flatnonzero(gates[:, e] > 0.0)
        if idx.size > NCAP:
            return None  # over capacity -> caller falls back to dense DP
        idxs.append(idx)
        xg = np.zeros((NCAP, D), np.float32)
        xg[: idx.size] = x[idx]
        grow = np.zeros((1, NCAP), np.float32)
        grow[0, : idx.size] = gates[idx, e]
        ep_maps.append(_make_ep_map(xg, grow, W1[e], b1[e], W2[e], b2[e], Wm[e]))

    # phase 2: one expert per core
    nc_e = _get_nc("ep")
    res_e = bass_utils.run_bass_kernel_spmd(nc_e, ep_maps, list(range(NCORES)))

    # combine on host (expert-ascending order, matching the reference sum)
    y = np.zeros((B, T), np.float32)
    for e in range(E):
        y[idxs[e]] += res_e.results[e]["yp"][: idxs[e].size].astype(np.float32)
    y[y == 0.0] = np.float32(EPS)
    return y


def kernel(x, labels, w_gate, W1, b1, W2, b2, Wm, _trace=False):
    x = np.asarray(x, dtype=np.float32)
    w_gate = np.asarray(w_gate, np.float32)
    W1 = np.asarray(W1, np.float32)
    b1 = np.asarray(b1, np.float32)
    W2 = np.asarray(W2, np.float32)
    b2 = np.asarray(b2, np.float32)
    Wm = np.asarray(Wm, np.float32)
    if MODE == "ep":
        y = _kernel_ep(x, w_gate, W1, b1, W2, b2, Wm)
        if y is not None:
            return y
        # pathological expert load imbalance: use the dense DP kernel
    in_maps = _host_prep(x, w_gate, W1, b1, W2, b2, Wm)
    nc = _get_nc()
    res = bass_utils.run_bass_kernel_spmd(
        nc, in_maps, list(range(NCORES)), trace=_trace
    )
    y = np.concatenate([res.results[c]["y"] for c in range(NCORES)], axis=0)
    if _trace:
        kernel.last_results = res
    return y
